# revision 5
# baseline (speedup 1.0000x reference)
"""Trainium2 Bass kernel for nn_MessageArMLP (GNN message passing), v4.

message[e, r, a, c] = node_feat[sender[e], r, a, c]
                      * sigmoid(rc[e] @ W[group(a)])[c] * cutoff[e]

Strategy (v4): nodes are sharded across the 8 cores (greedy degree
balance); each core's edges are sorted by sender and packed into
128-edge tiles.  Two consecutive tiles share one 32-slot node group
(dedup: a group's <=32 distinct senders are uploaded once, not per
tile), so the node stream is half the bytes of v3 and every input
stream is a full-128-partition DMA.  The gather is a PE matmul with a
one-hot P (cutoff folded in, bf16, loaded once for the whole core).
The decay logits use the same hi/lo bf16-split matmul as v3.  The
elementwise multiply is split three ways to balance engines: ACT
stages radial rows 0-2 (PSUM->bf16), DVE multiplies row 0 at 2x and
rows 3-7 directly from PSUM at 1x, and GpSimd (Pool) multiplies rows
1-2 from the staged bf16 (Pool has no PSUM port).  PSUM: ga(1 bank) +
gb(2) + dps(1), 2 bufs each = 8 banks exactly.
"""

import numpy as np
from contextlib import ExitStack

import ml_dtypes

import concourse.bass as bass
import concourse.tile as tile
from concourse import bacc, mybir
from concourse.bass_utils import run_bass_kernel_spmd

dt = mybir.dt
BF16 = ml_dtypes.bfloat16

# Problem constants (hardcoded per harness contract)
N_NODES = 10000
E_TOTAL = 120000
RADIAL = 8
ANG = 20
CH = 8
REMB = 8
ROW = RADIAL * ANG * CH     # 1280 elems per node row
ACOL = ANG * CH             # 160
G = 4
N_CORES = 8

T = 120                     # tiles per core (128 edges each)
NGRP = T // 2               # node groups (32 slots, 2 tiles each)
NBLK = (NGRP + 2) // 3      # node/P column blocks (3 groups per block; base 96 is illegal)
RBLK = (T + 2) // 3         # rct column blocks (3 tiles per block)
KDEC = 3 * REMB             # [rc_hi | rc_lo | rc_hi] x [W_hi | W_hi | W_lo]
PAIR = 2                    # tiles per output DMA
NCH = 1                     # node chunks per DMA (1 block = 8 tiles)
RCH = 10                    # rct blocks per DMA (30 tiles)

# elementwise split (cols of the 1280-wide radial x (ang,ch) row)
GA_W = 480                  # rows 0-2 -> ACT-staged
GB_W = 800                  # rows 3-7 -> DVE direct from PSUM
DVE_ST = 160                # staged row 0 -> DVE 2x
POOL_ST = 320               # staged rows 1-2 -> Pool

# angular groups for MAX_L=3: sizes 1,3,6,10 -> starts 0,1,4,10
GROUP_SLOTS = [(0, 1), (1, 3), (4, 6), (10, 10)]


def build_module():
    nc = bacc.Bacc(
        "TRN2",
        target_bir_lowering=False,
        debug=False,
        enable_asserts=False,
        num_devices=N_CORES,
    )
    ntab = nc.dram_tensor("ntab", [96, NBLK * ROW], dt.bfloat16, kind="ExternalInput").ap()
    pmat = nc.dram_tensor("pmat", [96, NBLK * 256], dt.bfloat16, kind="ExternalInput").ap()
    rct = nc.dram_tensor("rct", [96, RBLK * 128], dt.bfloat16, kind="ExternalInput").ap()
    wta = nc.dram_tensor("wta", [96, ACOL], dt.bfloat16, kind="ExternalInput").ap()
    msg = nc.dram_tensor("msg", [T * 128, ROW], dt.bfloat16, kind="ExternalOutput").ap()

    with tile.TileContext(nc) as tc:
        with ExitStack() as ctx:
            const_pool = ctx.enter_context(tc.tile_pool(name="const", bufs=1))
            node_pool = ctx.enter_context(tc.tile_pool(name="nodep", bufs=3))
            rct_pool = ctx.enter_context(tc.tile_pool(name="rctp", bufs=2))
            deca_pool = ctx.enter_context(tc.tile_pool(name="decap", bufs=4))
            cp_pool = ctx.enter_context(tc.tile_pool(name="cpp", bufs=4))
            out_pool = ctx.enter_context(tc.tile_pool(name="outp", bufs=6))
            ga_pool = ctx.enter_context(tc.tile_pool(name="ga", bufs=2, space="PSUM"))
            gb_pool = ctx.enter_context(tc.tile_pool(name="gb", bufs=2, space="PSUM"))
            dp_pool = ctx.enter_context(tc.tile_pool(name="dp", bufs=2, space="PSUM"))

            wta_sb = const_pool.tile([128, ACOL], dt.bfloat16)
            nc.scalar.dma_start(wta_sb[0:96, :], wta[:, :])
            p_sb = const_pool.tile([128, NBLK * 256], dt.bfloat16)
            nc.scalar.dma_start(p_sb[0:96, :], pmat[:, :])

            node_sbs = []
            rct_sbs = []
            out_sb = None

            def load_node(q):
                nsb = node_pool.tile([128, NCH * ROW], dt.bfloat16, tag="ntab", name="nsb")
                nc.scalar.dma_start(nsb[0:96, :], ntab[:, q * NCH * ROW : (q + 1) * NCH * ROW])
                node_sbs.append(nsb)

            def load_rct(ci):
                rsb = rct_pool.tile([128, RCH * 128], dt.bfloat16, tag="rct", name="rsb")
                nc.scalar.dma_start(rsb[0:96, :], rct[:, ci * RCH * 128 : (ci + 1) * RCH * 128])
                rct_sbs.append(rsb)

            load_rct(0)
            load_node(0)
            load_node(1)
            for t in range(T):
                j = t // 2          # node group
                q = j // 3          # node/P column block
                jb = 32 * (j % 3)   # node/P partition band
                tb = 32 * (t % 3)   # rct partition band

                # prefetch
                if t % 6 == 0 and t + 12 < T:
                    load_node(q + 2)
                if t % (RCH * 3) == RCH * 3 - 6 and t + 6 < T:
                    load_rct((t + 6) // (RCH * 3))

                node_sb = node_sbs[q]
                rct_sb = rct_sbs[t // (RCH * 3)]

                # decay logits: dps[e, ac] = sum_k rct[k, e] * wta[k, ac]
                dps = dp_pool.tile([128, ACOL], dt.float32, tag="dps")
                rc_off = (t // 3) % RCH * 128
                nc.tensor.matmul(
                    out=dps[:],
                    lhsT=rct_sb[tb : tb + KDEC, rc_off : rc_off + 128],
                    rhs=wta_sb[tb : tb + KDEC, :],
                    start=True,
                    stop=True,
                )
                deca = deca_pool.tile([128, ACOL], dt.bfloat16, tag="deca")
                nc.scalar.activation(
                    out=deca[:],
                    in_=dps[:],
                    func=mybir.ActivationFunctionType.Sigmoid,
                )

                # gather: psum[e, f] = sum_k P[k, e] * node[k, f]  (P carries cutoff)
                pm = p_sb[jb : jb + 32, q * 256 + (t % 2) * 128 : q * 256 + (t % 2) * 128 + 128]
                ga = ga_pool.tile([128, GA_W], dt.float32, tag="ga")
                gb = gb_pool.tile([128, GB_W], dt.float32, tag="gb")
                nc.tensor.matmul(
                    out=ga[:],
                    lhsT=pm,
                    rhs=node_sb[jb : jb + 32, 0:GA_W],
                    start=True,
                    stop=True,
                )
                nc.tensor.matmul(
                    out=gb[:, 0:512],
                    lhsT=pm,
                    rhs=node_sb[jb : jb + 32, GA_W : GA_W + 512],
                    start=True,
                    stop=True,
                )
                nc.tensor.matmul(
                    out=gb[:, 512:GB_W],
                    lhsT=pm,
                    rhs=node_sb[jb : jb + 32, GA_W + 512 : ROW],
                    start=True,
                    stop=True,
                )

                # out[e, r, a, c] = psum[e, r, ac] * deca[e, ac]
                p = t % PAIR
                if p == 0:
                    out_sb = out_pool.tile([128, PAIR, ROW], dt.bfloat16, tag="out")
                ov = out_sb[:, p, :]

                # rows 0-2 staged by ACT to bf16 SBUF
                cp = cp_pool.tile([128, GA_W], dt.bfloat16, tag="cp")
                nc.scalar.activation(
                    out=cp[:],
                    in_=ga[:],
                    func=mybir.ActivationFunctionType.Copy,
                )
                # row 0: DVE at 2x (all bf16 SBUF)
                nc.vector.tensor_mul(
                    out=ov[:, 0:DVE_ST],
                    in0=cp[:, 0:DVE_ST],
                    in1=deca[:],
                )
                # rows 1-2: Pool from staged bf16
                nc.gpsimd.tensor_mul(
                    out=ov[:, DVE_ST:GA_W].rearrange("p (r ac) -> p r ac", ac=ACOL),
                    in0=cp[:, DVE_ST:GA_W].rearrange("p (r ac) -> p r ac", ac=ACOL),
                    in1=deca[:].unsqueeze(1).to_broadcast([128, 2, ACOL]),
                )
                # rows 3-7: DVE direct from PSUM (1x)
                nc.vector.tensor_mul(
                    out=ov[:, GA_W:ROW].rearrange("p (r ac) -> p r ac", ac=ACOL),
                    in0=gb[:].rearrange("p (r ac) -> p r ac", ac=ACOL),
                    in1=deca[:].unsqueeze(1).to_broadcast([128, 5, ACOL]),
                )

                if p == PAIR - 1:
                    mv = msg[(t - 1) * 128 : (t + 1) * 128, :].rearrange(
                        "(j p) e -> p j e", j=PAIR
                    )
                    nc.sync.dma_start(out=mv, in_=out_sb[:])

    nc.compile()
    return nc


def _pack_core(eids, senders, cutoff, rc_all, node_bf16):
    """Sort a core's edges by sender; pack into 128-edge tiles where two
    consecutive tiles share one <=32-distinct-sender node group."""
    s = senders[eids]
    o = np.argsort(s, kind="stable")
    eids = eids[o]
    s = s[o]
    n = len(eids)
    uniq, pos = np.unique(s, return_inverse=True)

    # tiles: (edge_start, edge_end); groups: (node_base, node_count)
    tiles = []
    grp_of_tile = []
    grp_base = []
    i = 0
    while i < n:
        base = pos[i]
        lim = np.searchsorted(pos, base + 32, side="left")
        g = len(grp_base)
        grp_base.append(base)
        for _ in range(2):
            if i >= n or i >= lim:
                break
            j = min(i + 128, lim, n)
            tiles.append((i, j))
            grp_of_tile.append(g)
            i = j
        # enforce fixed schedule: group g must own tiles 2g, 2g+1
        while len(tiles) < 2 * (g + 1):
            tiles.append((i, i))
            grp_of_tile.append(g)

    nt = len(tiles)
    ng = len(grp_base)
    assert nt <= T, f"tile capacity exceeded: {nt} > {T}"
    assert ng <= NGRP, f"group capacity exceeded: {ng} > {NGRP}"

    # node table [96, NBLK*ROW]: group g -> partitions 32*(g%3), col block g//3
    slot_nid = np.zeros((NBLK, 3, 32), np.int64)
    slot_valid = np.zeros((NBLK, 3, 32), bool)
    for g, base in enumerate(grp_base):
        hi = grp_base[g + 1] if g + 1 < ng else len(uniq)
        cnt = min(hi - base, 32)
        # group may end early if tiles were node-limited; cnt from tiles:
        e0 = tiles[2 * g][0]
        e1 = tiles[2 * g + 1][1]
        if e1 > e0:
            cnt = pos[e1 - 1] - base + 1
        else:
            cnt = 0
        slot_nid[g // 3, g % 3, :cnt] = uniq[base : base + cnt]
        slot_valid[g // 3, g % 3, :cnt] = True

    ntab = np.zeros((NBLK, 3, 32, ROW), BF16)
    ntab[slot_valid] = node_bf16[slot_nid[slot_valid]]
    ntab = np.ascontiguousarray(
        ntab.transpose(1, 2, 0, 3).reshape(96, NBLK * ROW)
    )

    # P [96, NBLK*256]: tile t -> partitions 32*(g%3)+slot, col q*256+(t%2)*128+idx
    pmat = np.zeros((3, 32, NBLK, 256), np.float32)
    # rct [96, RBLK*128]: tile t -> partitions 32*(t%3)+k, col (t//3)*128+idx
    rctm = np.zeros((3, 32, RBLK, 128), np.float32)
    eid_map = np.full((T, 128), -1, np.int64)

    for t, (i0, i1) in enumerate(tiles):
        if i1 <= i0:
            continue
        g = grp_of_tile[t]
        te = eids[i0:i1]
        w = i1 - i0
        k = pos[i0:i1] - grp_base[g]
        idx = np.arange(w)
        pmat[g % 3, k, g // 3, (t % 2) * 128 + idx] = cutoff[te]
        rc = rc_all[te]  # [w, 8] f32
        hi = rc.astype(BF16).astype(np.float32)
        lo = rc - hi
        stack = np.concatenate([hi, lo, hi], axis=1)  # [w, 24]
        rctm[t % 3, 0:KDEC, t // 3, idx] = stack
        eid_map[t, :w] = te

    return {
        "ntab": ntab,
        "pmat": np.ascontiguousarray(pmat.reshape(96, NBLK * 256).astype(BF16)),
        "rct": np.ascontiguousarray(rctm.reshape(96, RBLK * 128).astype(BF16)),
    }, eid_map.reshape(-1)


def make_in_maps(node_feat, radial_component, radial_cutoff_fn, weights, edge_index):
    node_flat = np.asarray(node_feat, dtype=np.float32).reshape(N_NODES, ROW)
    node_bf16 = node_flat.astype(BF16)
    w = np.asarray(weights, dtype=np.float32)  # [G, REMB, CH]
    wtf = np.zeros((REMB, ACOL), np.float32)
    for g, (s0, ns) in enumerate(GROUP_SLOTS):
        for a in range(s0, s0 + ns):
            wtf[:, a * CH : (a + 1) * CH] = w[g]
    w_hi = wtf.astype(BF16).astype(np.float32)
    w_lo = wtf - w_hi
    wrows = np.concatenate([w_hi, w_hi, w_lo], axis=0)  # [24, 160]
    wta = np.zeros((3, 32, ACOL), np.float32)
    wta[:, 0:KDEC, :] = wrows[None]
    wta = np.ascontiguousarray(wta.reshape(96, ACOL).astype(BF16))

    senders = np.asarray(edge_index)[0].astype(np.int64)
    rc_all = np.asarray(radial_component, dtype=np.float32)
    cut_all = np.asarray(radial_cutoff_fn, dtype=np.float32)

    deg = np.bincount(senders, minlength=N_NODES)
    order = np.argsort(-deg, kind="stable")
    node_core = np.empty(N_NODES, np.int32)
    import heapq

    heap = [(0, c) for c in range(N_CORES)]
    heapq.heapify(heap)
    for nd in order:
        load, c = heapq.heappop(heap)
        node_core[nd] = c
        heapq.heappush(heap, (load + int(deg[nd]), c))

    edge_core = node_core[senders]
    in_maps, eid_maps = [], []
    for c in range(N_CORES):
        eids = np.nonzero(edge_core == c)[0]
        m, emap = _pack_core(eids, senders, cut_all, rc_all, node_bf16)
        m["wta"] = wta
        in_maps.append(m)
        eid_maps.append(emap)
    return in_maps, eid_maps


def assemble(results, eid_maps):
    out = np.empty((E_TOTAL, ROW), np.float32)
    for r, emap in zip(results, eid_maps):
        valid = emap >= 0
        m = np.asarray(r["msg"]).reshape(T * 128, ROW)[valid]
        f32 = (m.view(np.uint16).astype(np.uint32) << np.uint32(16)).view(np.float32)
        out[emap[valid]] = f32
    return out.reshape(E_TOTAL, RADIAL, ANG, CH)


_nc_cache = None


def _get_module():
    global _nc_cache
    if _nc_cache is None:
        _nc_cache = build_module()
    return _nc_cache


def kernel(node_feat, radial_component, radial_cutoff_fn, weights, edge_index):
    nc = _get_module()
    in_maps, eid_maps = make_in_maps(
        node_feat, radial_component, radial_cutoff_fn, weights, edge_index
    )
    res = run_bass_kernel_spmd(nc, in_maps, core_ids=list(range(N_CORES)))
    return assemble(res.results, eid_maps)


# revision 9
# speedup vs baseline: 1.0153x; 1.0153x over previous
"""Trainium2 Bass kernel for nn_MessageArMLP (GNN message passing), v6.

message[e, r, a, c] = node_feat[sender[e], r, a, c]
                      * sigmoid(rc[e] @ W[group(a)])[c] * cutoff[e]

Strategy (v6): nodes are sharded across the 8 cores (greedy degree
balance); each core's edges are sorted by sender and packed into
128-edge tiles.  Two consecutive tiles share one 32-slot node group
(dedup: a group's <=32 distinct senders are uploaded once, not per
tile).  Groups cycle through partition bands {0,32,64} (base-96 PE
operands are illegal, and K=64 at base 64 hangs the device), so the
node/P/rct streams use partitions 0-95.  The gather is a K=32 PE
matmul with a one-hot P (cutoff folded in, bf16).  Decay logits batch
2 tiles into one PSUM tile and one sigmoid.  The elementwise multiply
is rebalanced to unload DVE: ACT stages radial rows 0-4 (PSUM->bf16,
one 800-col op), DVE multiplies rows 0-2 at 2x from the staged bf16
and rows 5-7 directly from PSUM at 1x, Pool (GpSimd, no PSUM port)
multiplies rows 3-4 from the staged bf16.  PSUM: ga(2 banks) + gb(1) +
dps(1), 2 bufs each = 8 banks exactly.
"""

import numpy as np
from contextlib import ExitStack

import ml_dtypes

import concourse.bass as bass
import concourse.tile as tile
from concourse import bacc, mybir
from concourse.bass_utils import run_bass_kernel_spmd

dt = mybir.dt
BF16 = ml_dtypes.bfloat16

# Problem constants (hardcoded per harness contract)
N_NODES = 10000
E_TOTAL = 120000
RADIAL = 8
ANG = 20
CH = 8
REMB = 8
ROW = RADIAL * ANG * CH     # 1280 elems per node row
ACOL = ANG * CH             # 160
G = 4
N_CORES = 8

T = 120                     # tiles per core (128 edges each)
NGRP = T // 2               # node groups (32 slots, 2 tiles each)
NBLK = (NGRP + 2) // 3      # node/P column blocks (3 groups per block)
RBLK = (T + 2) // 3         # rct column blocks (3 tiles per block)
KDEC = 3 * REMB             # [rc_hi | rc_lo | rc_hi] x [W_hi | W_hi | W_lo]
PAIR = 2                    # tiles per output DMA
RCH = 10                    # rct blocks per DMA chunk (30 tiles)

# elementwise split
GA_W = 800                  # rows 0-4: ACT-staged (2 PSUM banks)
GB_W = 480                  # rows 5-7: DVE direct from PSUM (1 bank)
DVE_ST = 480                # staged rows 0-2 -> DVE 2x
POOL_ST = 320               # staged rows 3-4 -> Pool
DP_W = 160                  # per-tile decay logits (one matmul per PSUM bank)

# angular groups for MAX_L=3: sizes 1,3,6,10 -> starts 0,1,4,10
GROUP_SLOTS = [(0, 1), (1, 3), (4, 6), (10, 10)]


def build_module():
    nc = bacc.Bacc(
        "TRN2",
        target_bir_lowering=False,
        debug=False,
        enable_asserts=False,
        num_devices=N_CORES,
    )
    ntab = nc.dram_tensor("ntab", [96, NBLK * ROW], dt.bfloat16, kind="ExternalInput").ap()
    pmat = nc.dram_tensor("pmat", [96, NBLK * 256], dt.bfloat16, kind="ExternalInput").ap()
    rct = nc.dram_tensor("rct", [96, RBLK * 128], dt.bfloat16, kind="ExternalInput").ap()
    wta = nc.dram_tensor("wta", [96, ACOL], dt.bfloat16, kind="ExternalInput").ap()
    msg = nc.dram_tensor("msg", [T * 128, ROW], dt.bfloat16, kind="ExternalOutput").ap()

    with tile.TileContext(nc) as tc:
        with ExitStack() as ctx:
            const_pool = ctx.enter_context(tc.tile_pool(name="const", bufs=1))
            node_pool = ctx.enter_context(tc.tile_pool(name="nodep", bufs=3))
            p_pool = ctx.enter_context(tc.tile_pool(name="pp", bufs=3))
            rct_pool = ctx.enter_context(tc.tile_pool(name="rctp", bufs=2))
            deca_pool = ctx.enter_context(tc.tile_pool(name="decap", bufs=4))
            cp_pool = ctx.enter_context(tc.tile_pool(name="cpp", bufs=4))
            out_pool = ctx.enter_context(tc.tile_pool(name="outp", bufs=6))
            ga_pool = ctx.enter_context(tc.tile_pool(name="ga", bufs=2, space="PSUM"))
            gb_pool = ctx.enter_context(tc.tile_pool(name="gb", bufs=2, space="PSUM"))
            dp_pool = ctx.enter_context(tc.tile_pool(name="dp", bufs=2, space="PSUM"))

            wta_sb = const_pool.tile([128, ACOL], dt.bfloat16)
            nc.sync.dma_start(wta_sb[0:96, :], wta[:, :])

            node_sbs = []
            p_sbs = []
            rct_sbs = []
            out_sb = None
            deca2 = None

            def load_node(q):
                nsb = node_pool.tile([128, ROW], dt.bfloat16, tag="ntab", name="nsb")
                nc.scalar.dma_start(nsb[0:96, :], ntab[:, q * ROW : (q + 1) * ROW])
                node_sbs.append(nsb)
                psb = p_pool.tile([128, 256], dt.bfloat16, tag="pmat", name="psb")
                nc.scalar.dma_start(psb[0:96, :], pmat[:, q * 256 : (q + 1) * 256])
                p_sbs.append(psb)

            def load_rct(ci):
                rsb = rct_pool.tile([128, RCH * 128], dt.bfloat16, tag="rct", name="rsb")
                nc.scalar.dma_start(rsb[0:96, :], rct[:, ci * RCH * 128 : (ci + 1) * RCH * 128])
                rct_sbs.append(rsb)

            load_rct(0)
            load_node(0)
            load_node(1)
            for t in range(T):
                j = t // 2            # node group
                q = j // 3            # node/P column block
                jb = 32 * (j % 3)     # node/P partition band

                # prefetch
                if t % 6 == 0 and t + 12 < T:
                    load_node(q + 2)
                if t % (RCH * 3) == RCH * 3 - 6 and t + 6 < T:
                    load_rct((t + 6) // (RCH * 3))

                node_sb = node_sbs[q]
                p_sb = p_sbs[q]
                rct_sb = rct_sbs[t // (RCH * 3)]

                # decay logits per tile (one matmul, one bank) + sigmoid
                tb = 32 * (t % 3)
                rc_off = (t // 3) % RCH * 128
                dps = dp_pool.tile([128, DP_W], dt.float32, tag="dps")
                nc.tensor.matmul(
                    out=dps[:],
                    lhsT=rct_sb[tb : tb + KDEC, rc_off : rc_off + 128],
                    rhs=wta_sb[tb : tb + KDEC, :],
                    start=True,
                    stop=True,
                )
                deca = deca_pool.tile([128, ACOL], dt.bfloat16, tag="deca")
                nc.scalar.activation(
                    out=deca[:],
                    in_=dps[:],
                    func=mybir.ActivationFunctionType.Sigmoid,
                )

                # gather: psum[e, f] = sum_k P[k, e] * node[k, f]  (P carries cutoff)
                pm = p_sb[jb : jb + 32, (t % 2) * 128 : (t % 2) * 128 + 128]
                ga = ga_pool.tile([128, GA_W], dt.float32, tag="ga")
                gb = gb_pool.tile([128, GB_W], dt.float32, tag="gb")
                nc.tensor.matmul(
                    out=ga[:, 0:512],
                    lhsT=pm,
                    rhs=node_sb[jb : jb + 32, 0:512],
                    start=True,
                    stop=True,
                )
                nc.tensor.matmul(
                    out=ga[:, 512:GA_W],
                    lhsT=pm,
                    rhs=node_sb[jb : jb + 32, 512:GA_W],
                    start=True,
                    stop=True,
                )
                nc.tensor.matmul(
                    out=gb[:],
                    lhsT=pm,
                    rhs=node_sb[jb : jb + 32, GA_W:ROW],
                    start=True,
                    stop=True,
                )

                # out[e, r, a, c] = psum[e, r, ac] * deca[e, ac]
                p = t % PAIR
                if p == 0:
                    out_sb = out_pool.tile([128, PAIR, ROW], dt.bfloat16, tag="out")
                ov = out_sb[:, p, :]

                # rows 0-4 staged by ACT to bf16 SBUF (one op)
                cp = cp_pool.tile([128, GA_W], dt.bfloat16, tag="cp")
                nc.scalar.activation(
                    out=cp[:],
                    in_=ga[:],
                    func=mybir.ActivationFunctionType.Copy,
                )
                # rows 0-2: DVE at 2x (all bf16 SBUF)
                nc.vector.tensor_mul(
                    out=ov[:, 0:DVE_ST].rearrange("p (r ac) -> p r ac", ac=ACOL),
                    in0=cp[:, 0:DVE_ST].rearrange("p (r ac) -> p r ac", ac=ACOL),
                    in1=deca.unsqueeze(1).to_broadcast([128, 3, ACOL]),
                )
                # rows 3-4: Pool from staged bf16
                nc.gpsimd.tensor_mul(
                    out=ov[:, DVE_ST:GA_W].rearrange("p (r ac) -> p r ac", ac=ACOL),
                    in0=cp[:, DVE_ST:GA_W].rearrange("p (r ac) -> p r ac", ac=ACOL),
                    in1=deca.unsqueeze(1).to_broadcast([128, 2, ACOL]),
                )
                # rows 5-7: DVE direct from PSUM (1x)
                nc.vector.tensor_mul(
                    out=ov[:, GA_W:ROW].rearrange("p (r ac) -> p r ac", ac=ACOL),
                    in0=gb[:].rearrange("p (r ac) -> p r ac", ac=ACOL),
                    in1=deca.unsqueeze(1).to_broadcast([128, 3, ACOL]),
                )

                if p == PAIR - 1:
                    mv = msg[(t - 1) * 128 : (t + 1) * 128, :].rearrange(
                        "(j p) e -> p j e", j=PAIR
                    )
                    nc.sync.dma_start(out=mv, in_=out_sb[:])

    nc.compile()
    return nc


def _pack_core(eids, senders, cutoff, rc_all, node_bf16):
    """Sort a core's edges by sender; pack into 128-edge tiles where two
    consecutive tiles share one <=32-distinct-sender node group."""
    s = senders[eids]
    o = np.argsort(s, kind="stable")
    eids = eids[o]
    s = s[o]
    n = len(eids)
    uniq, pos = np.unique(s, return_inverse=True)

    tiles = []
    grp_base = []
    i = 0
    while i < n:
        base = pos[i]
        lim = np.searchsorted(pos, base + 32, side="left")
        g = len(grp_base)
        grp_base.append(base)
        for _ in range(2):
            if i >= n or i >= lim:
                break
            j = min(i + 128, lim, n)
            tiles.append((i, j))
            i = j
        while len(tiles) < 2 * (g + 1):
            tiles.append((i, i))

    nt = len(tiles)
    ng = len(grp_base)
    assert nt <= T, f"tile capacity exceeded: {nt} > {T}"
    assert ng <= NGRP, f"group capacity exceeded: {ng} > {NGRP}"

    # node table [96, NBLK*ROW]: group g -> partitions 32*(g%3), col block g//3
    ntab = np.zeros((96, NBLK * ROW), BF16)
    for g, base in enumerate(grp_base):
        e0 = tiles[2 * g][0]
        e1 = tiles[2 * g + 1][1]
        cnt = pos[e1 - 1] - base + 1 if e1 > e0 else 0
        if cnt == 0:
            continue
        r0 = 32 * (g % 3)
        c0 = (g // 3) * ROW
        ntab[r0 : r0 + cnt, c0 : c0 + ROW] = node_bf16[uniq[base : base + cnt]]

    pmat = np.zeros((96, NBLK * 256), np.float32)
    rctm = np.zeros((96, RBLK * 128), np.float32)
    eid_map = np.full((T, 128), -1, np.int64)

    for t, (i0, i1) in enumerate(tiles):
        if i1 <= i0:
            continue
        g = t // 2
        te = eids[i0:i1]
        w = i1 - i0
        k = pos[i0:i1] - grp_base[g]
        idx = np.arange(w)
        pmat[32 * (g % 3) + k, (g // 3) * 256 + (t % 2) * 128 + idx] = cutoff[te]
        rc = rc_all[te]  # [w, 8] f32
        hi = rc.astype(BF16).astype(np.float32)
        lo = rc - hi
        stack = np.concatenate([hi, lo, hi], axis=1)  # [w, 24]
        band = 32 * (t % 3)
        rctm[band : band + KDEC, (t // 3) * 128 + idx] = stack.T
        eid_map[t, :w] = te

    return {
        "ntab": ntab,
        "pmat": np.ascontiguousarray(pmat.astype(BF16)),
        "rct": np.ascontiguousarray(rctm.astype(BF16)),
    }, eid_map.reshape(-1)


def make_in_maps(node_feat, radial_component, radial_cutoff_fn, weights, edge_index):
    node_flat = np.asarray(node_feat, dtype=np.float32).reshape(N_NODES, ROW)
    node_bf16 = node_flat.astype(BF16)
    w = np.asarray(weights, dtype=np.float32)  # [G, REMB, CH]
    wtf = np.zeros((REMB, ACOL), np.float32)
    for g, (s0, ns) in enumerate(GROUP_SLOTS):
        for a in range(s0, s0 + ns):
            wtf[:, a * CH : (a + 1) * CH] = w[g]
    w_hi = wtf.astype(BF16).astype(np.float32)
    w_lo = wtf - w_hi
    wrows = np.concatenate([w_hi, w_hi, w_lo], axis=0)  # [24, 160]
    wta = np.zeros((96, ACOL), np.float32)
    for b in range(3):
        wta[32 * b : 32 * b + KDEC, :] = wrows
    wta = np.ascontiguousarray(wta.astype(BF16))

    senders = np.asarray(edge_index)[0].astype(np.int64)
    rc_all = np.asarray(radial_component, dtype=np.float32)
    cut_all = np.asarray(radial_cutoff_fn, dtype=np.float32)

    deg = np.bincount(senders, minlength=N_NODES)
    order = np.argsort(-deg, kind="stable")
    node_core = np.empty(N_NODES, np.int32)
    import heapq

    heap = [(0, c) for c in range(N_CORES)]
    heapq.heapify(heap)
    for nd in order:
        load, c = heapq.heappop(heap)
        node_core[nd] = c
        heapq.heappush(heap, (load + int(deg[nd]), c))

    edge_core = node_core[senders]
    in_maps, eid_maps = [], []
    for c in range(N_CORES):
        eids = np.nonzero(edge_core == c)[0]
        m, emap = _pack_core(eids, senders, cut_all, rc_all, node_bf16)
        m["wta"] = wta
        in_maps.append(m)
        eid_maps.append(emap)
    return in_maps, eid_maps


def assemble(results, eid_maps):
    out = np.empty((E_TOTAL, ROW), np.float32)
    for r, emap in zip(results, eid_maps):
        valid = emap >= 0
        m = np.asarray(r["msg"]).reshape(T * 128, ROW)[valid]
        f32 = (m.view(np.uint16).astype(np.uint32) << np.uint32(16)).view(np.float32)
        out[emap[valid]] = f32
    return out.reshape(E_TOTAL, RADIAL, ANG, CH)


_nc_cache = None


def _get_module():
    global _nc_cache
    if _nc_cache is None:
        _nc_cache = build_module()
    return _nc_cache


def kernel(node_feat, radial_component, radial_cutoff_fn, weights, edge_index):
    nc = _get_module()
    in_maps, eid_maps = make_in_maps(
        node_feat, radial_component, radial_cutoff_fn, weights, edge_index
    )
    res = run_bass_kernel_spmd(nc, in_maps, core_ids=list(range(N_CORES)))
    return assemble(res.results, eid_maps)
